# revision 2
# baseline (speedup 1.0000x reference)
"""MoE top-2 routing kernel for Trainium2 (8 NeuronCores, expert-parallel).

Strategy
--------
Host (cheap, 16384x4-sized math): router logits, sinkhorn, top-2 indices and
sigmoid gates — computed with jax on CPU, replicating the reference bitwise.
Tokens are permuted by expert on the host while sharding: each of the 8 cores
owns half of one expert's (token, gate) list plus that expert's W1/W2 (bf16).

Device (the heavy ~17 GFLOP/core): dense FFN over the pre-gathered tokens in
feature-major layout, weight-stationary matmuls from SBUF:
    h1T = silu(W1_chunk.T @ xT)      [F-major]
    y   = gate * (h1T_chunk.T @ W2)  [token-major out]
Host scatter-adds the two expert contributions per token (no duplicates per
core, so fancy-index += is safe).
"""
import sys
import types

import numpy as np
import ml_dtypes

H = 512
F = 2048
E = 4
P = 128
PANEL = 512
NCORES = 8
T_TOTAL = 16384
DEFAULT_C = 4608  # rows (token,expert pairs) per core, multiple of PANEL

_BF16 = ml_dtypes.bfloat16


# ---------------------------------------------------------------------------
# compat shims (axon image): NTFF hook module + core_v3 drain-wait splitting
# ---------------------------------------------------------------------------
_COMPAT_DONE = False


def _install_compat():
    global _COMPAT_DONE
    if _COMPAT_DONE:
        return
    if "antenv.axon_hooks" not in sys.modules:
        mod = types.ModuleType("antenv.axon_hooks")
        try:
            from trn_agent_boot.trn_boot import _ntff_profile_via_ctypes
            _hook = _ntff_profile_via_ctypes("/opt/axon/libaxon_pjrt.so")
        except Exception:
            _hook = None
        mod.get_axon_ntff_profile_hook = lambda: _hook
        mod.set_axon_ntff_profile_hook = lambda h: None
        sys.modules["antenv.axon_hooks"] = mod

    import concourse.mybir as mybir
    import concourse.tile as tile
    from bass_rust import VectorClock, ScopedClock, N_PROCS

    if not getattr(tile.TileContext._add_instruction, "_split_waits_patch", False):
        # This walrus build accepts at most ONE sync wait per instruction
        # ("Too many sync wait commands"). Split extras onto single-wait
        # nops on the same engine, inserted immediately before. Stalling the
        # engine at the same program point is strictly stronger than the
        # per-instruction wait, and every waited-on producer is issued
        # earlier in program order, so this cannot deadlock.
        _orig_add = tile.TileContext._add_instruction

        def _add_instruction(self, inst):
            si = inst.sync_info
            if si is not None and si.on_wait and len(si.on_wait) > 1:
                waits = list(si.on_wait)
                for w in waits[:-1]:
                    nop = mybir.InstNoOp(
                        name=self.nc.get_next_instruction_name()
                    )
                    nop.engine = inst.engine
                    nop.sync_info = mybir.SyncInfo(on_wait=[w], on_update=[])
                    _orig_add(self, nop)
                inst.sync_info = mybir.SyncInfo(
                    on_wait=[waits[-1]], on_update=list(si.on_update or [])
                )
            _orig_add(self, inst)

        _add_instruction._split_waits_patch = True
        tile.TileContext._add_instruction = _add_instruction

    if not getattr(tile.TileContext._drain_and_barrier, "_split_waits_patch", False):
        # core_v3 walrus rejects a Drain carrying >1 sync wait ("Too many sync
        # wait commands"); put each wait on its own in-order SP nop instead.
        def _drain_and_barrier(self, tick_clock, wait_clock):
            nc = self.nc
            gc = tick_clock.global_clock
            for p in range(N_PROCS):
                t = gc[p]
                if t == 0:
                    continue
                vc = VectorClock([t if i == p else 0 for i in range(N_PROCS)])
                n = nc.sync.nop()
                wait_clock.add_sem_waits(n.ins, ScopedClock({None: vc}))
            nc.sync.drain()
            nc.all_engine_barrier()
            popped = nc._tile_sem_poison_stack.pop()
            assert popped is self._sem_poison
            nc.clear_and_free_semaphores(list(self.sems.allocated().values()))
            nc.all_engine_barrier()

        _drain_and_barrier._split_waits_patch = True
        tile.TileContext._drain_and_barrier = _drain_and_barrier

    from concourse import bass_utils
    bass_utils.upload_artifacts = lambda tmpdir: tmpdir
    _COMPAT_DONE = True


# ---------------------------------------------------------------------------
# host routing — bitwise replication of the reference (jax on CPU)
# ---------------------------------------------------------------------------
def _route(xt_f32, w_router):
    import jax
    import jax.numpy as jnp
    from jax import lax

    cpu = jax.devices("cpu")[0]

    def sinkhorn(cost, tol=1e-4):
        cost = jnp.exp(cost)
        T, E_ = cost.shape
        eps = 1e-8

        def cond(state):
            _, _, err = state
            return err > tol

        def body(state):
            d0, d1, _ = state
            d0n = (1.0 / T) / (cost @ d1 + eps)
            d1n = (1.0 / E_) / (d0n @ cost + eps)
            return (d0n, d1n, jnp.mean(jnp.abs(d1 - d1n)))

        init = (jnp.ones((T,), cost.dtype), jnp.ones((E_,), cost.dtype),
                jnp.asarray(1e9, cost.dtype))
        d0, d1, _ = lax.while_loop(cond, body, init)
        return d1 * cost * d0[:, None]

    with jax.default_device(cpu):
        xt_j = jnp.asarray(xt_f32)
        logits = xt_j @ jnp.asarray(w_router)
        norm = sinkhorn(logits.astype(jnp.float32))
        _, indices = lax.top_k(norm, 2)
        scores = jnp.take_along_axis(jax.nn.sigmoid(logits), indices, axis=1)
        return np.asarray(indices), np.asarray(scores)


# ---------------------------------------------------------------------------
# device kernel
# ---------------------------------------------------------------------------
_BUILD_CACHE = {}
LAST_EXEC_NS = None
LAST_RESULTS = None


def _build_nc(C):
    """Bass program for one core: dense FFN over C pre-gathered rows."""
    import concourse.bass as bass
    import concourse.mybir as mybir
    import concourse.tile as tile

    NP = C // PANEL        # token panels
    KC = H // P            # 4  k-chunks over hidden
    FC = F // P            # 16 f-chunks over ffn
    TCH = PANEL // P       # 4  token chunks per panel
    bf16 = mybir.dt.bfloat16
    f32 = mybir.dt.float32

    nc = bass.Bass()
    xt_d = nc.dram_tensor("xt", [P, KC, C], bf16, kind="ExternalInput")
    w1_d = nc.dram_tensor("w1", [P, KC, F], bf16, kind="ExternalInput")
    w2_d = nc.dram_tensor("w2", [P, FC, H], bf16, kind="ExternalInput")
    g_d = nc.dram_tensor("g", [P, C // P], f32, kind="ExternalInput")
    y_d = nc.dram_tensor("y", [C // P, P, H], f32, kind="ExternalOutput")

    with tile.TileContext(nc) as tc:
        with (
            tc.tile_pool(name="wpool", bufs=1) as wp,
            tc.tile_pool(name="xpool", bufs=3) as xp,
            tc.tile_pool(name="hpool", bufs=2) as hp,
            tc.tile_pool(name="opool", bufs=4) as op,
            tc.tile_pool(name="psum", bufs=4, space="PSUM") as pp,
        ):
            w1_sb = wp.tile([P, KC, F], bf16)
            nc.sync.dma_start(out=w1_sb, in_=w1_d[:, :, :])
            w2_sb = wp.tile([P, FC, H], bf16)
            nc.sync.dma_start(out=w2_sb, in_=w2_d[:, :, :])
            g_sb = wp.tile([P, C // P], f32)
            nc.sync.dma_start(out=g_sb, in_=g_d[:, :])

            for ip in range(NP):
                x_sb = xp.tile([P, KC, PANEL], bf16, tag="x")
                nc.sync.dma_start(
                    out=x_sb, in_=xt_d[:, :, ip * PANEL:(ip + 1) * PANEL]
                )
                h1_sb = hp.tile([P, FC, PANEL], bf16, tag="h1")
                for fc in range(FC):
                    ps = pp.tile([P, PANEL], f32, tag="ps1")
                    for kc in range(KC):
                        nc.tensor.matmul(
                            ps,
                            w1_sb[:, kc, fc * P:(fc + 1) * P],
                            x_sb[:, kc, :],
                            start=(kc == 0),
                            stop=(kc == KC - 1),
                        )
                    nc.scalar.activation(
                        out=h1_sb[:, fc, :], in_=ps,
                        func=mybir.ActivationFunctionType.Silu,
                    )
                for tch in range(TCH):
                    ps2 = pp.tile([P, H], f32, tag="ps2")
                    for fc in range(FC):
                        nc.tensor.matmul(
                            ps2,
                            h1_sb[:, fc, tch * P:(tch + 1) * P],
                            w2_sb[:, fc, :],
                            start=(fc == 0),
                            stop=(fc == FC - 1),
                        )
                    o_sb = op.tile([P, H], f32, tag="o")
                    j = ip * TCH + tch
                    nc.vector.tensor_scalar_mul(o_sb, ps2, g_sb[:, j:j + 1])
                    nc.sync.dma_start(out=y_d[j], in_=o_sb)
    return nc


def _pack_core(xt_f32, toks, gates, w1_e_bf, w2_e_bf, C):
    n = len(toks)
    xr = np.zeros((C, H), _BF16)
    xr[:n] = xt_f32[toks].astype(_BF16)
    # [C,H] -> [H,C] -> [KC,P,C] -> [P,KC,C]
    xt_pack = np.ascontiguousarray(
        xr.T.reshape(H // P, P, C).transpose(1, 0, 2)
    )
    g = np.zeros((C,), np.float32)
    g[:n] = gates
    g_pack = np.ascontiguousarray(g.reshape(C // P, P).T)
    return {"xt": xt_pack, "w1": w1_e_bf, "w2": w2_e_bf, "g": g_pack}


def kernel(input, w_router, w1, w2):
    global LAST_EXEC_NS, LAST_RESULTS
    import os

    _install_compat()
    from concourse.bass_utils import run_bass_kernel_spmd

    x = np.asarray(input, dtype=np.float32)
    w_router = np.asarray(w_router, dtype=np.float32)
    w1 = np.asarray(w1, dtype=np.float32)
    w2 = np.asarray(w2, dtype=np.float32)
    s, b, h = x.shape
    T = s * b
    xt = np.ascontiguousarray(x.reshape(T, h))

    indices, scores = _route(xt, w_router)

    # per-expert (token, gate) lists
    tok_lists = []
    gate_lists = []
    for e in range(E):
        toks = []
        gs = []
        for k in range(2):
            sel = np.nonzero(indices[:, k] == e)[0]
            toks.append(sel)
            gs.append(scores[sel, k])
        tok_lists.append(np.concatenate(toks))
        gate_lists.append(np.concatenate(gs).astype(np.float32))

    max_half = max((len(t) + 1) // 2 for t in tok_lists)
    C = max(DEFAULT_C, ((max_half + PANEL - 1) // PANEL) * PANEL)

    if C not in _BUILD_CACHE:
        _BUILD_CACHE[C] = _build_nc(C)
    nc = _BUILD_CACHE[C]

    # weights per expert, packed [P, KC, F] / [P, FC, H] bf16
    w1_packs = [
        np.ascontiguousarray(
            w1[e].astype(_BF16).reshape(H // P, P, F).transpose(1, 0, 2)
        )
        for e in range(E)
    ]
    w2_packs = [
        np.ascontiguousarray(
            w2[e].astype(_BF16).reshape(F // P, P, H).transpose(1, 0, 2)
        )
        for e in range(E)
    ]

    in_maps = []
    core_toks = []
    for c in range(NCORES):
        e = c // 2
        toks_e = tok_lists[e]
        gates_e = gate_lists[e]
        half = (len(toks_e) + 1) // 2
        if c % 2 == 0:
            toks, gs = toks_e[:half], gates_e[:half]
        else:
            toks, gs = toks_e[half:], gates_e[half:]
        assert len(toks) <= C
        core_toks.append(toks)
        in_maps.append(_pack_core(xt, toks, gs, w1_packs[e], w2_packs[e], C))

    trace = bool(int(os.environ.get("BASS_MOE_TRACE", "0")))
    res = run_bass_kernel_spmd(nc, in_maps, list(range(NCORES)), trace=trace)
    LAST_EXEC_NS = res.exec_time_ns
    LAST_RESULTS = res

    out = np.zeros((T, H), np.float32)
    for c in range(NCORES):
        y = res.results[c]["y"].reshape(-1, H)
        toks = core_toks[c]
        out[toks] += y[: len(toks)]
    return out.reshape(s, b, h)


# revision 5
# speedup vs baseline: 1.1234x; 1.1234x over previous
"""MoE top-2 routing kernel for Trainium2 (8 NeuronCores, expert-parallel).

Strategy
--------
Host (cheap, 16384x4-sized math): router logits, sinkhorn, top-2 indices and
sigmoid gates — computed with jax on CPU, replicating the reference bitwise.
Tokens are permuted by expert on the host while sharding: each of the 8 cores
owns half of one expert's (token, gate) list plus that expert's W1/W2 (bf16).

Device (the heavy ~17 GFLOP/core): dense FFN over the pre-gathered tokens in
feature-major layout, weight-stationary matmuls from SBUF:
    h1T = silu(W1_chunk.T @ xT)      [F-major]
    y   = gate * (h1T_chunk.T @ W2)  [token-major out]
Host scatter-adds the two expert contributions per token (no duplicates per
core, so fancy-index += is safe).
"""
import sys
import types

import numpy as np
import ml_dtypes

H = 512
F = 2048
E = 4
P = 128
PANEL = 512
NCORES = 8
T_TOTAL = 16384
DEFAULT_C = 4224  # rows (token,expert pairs) per core, multiple of 128

_BF16 = ml_dtypes.bfloat16


# ---------------------------------------------------------------------------
# compat shims (axon image): NTFF hook module + core_v3 drain-wait splitting
# ---------------------------------------------------------------------------
_COMPAT_DONE = False


def _install_compat():
    global _COMPAT_DONE
    if _COMPAT_DONE:
        return
    if "antenv.axon_hooks" not in sys.modules:
        mod = types.ModuleType("antenv.axon_hooks")
        try:
            from trn_agent_boot.trn_boot import _ntff_profile_via_ctypes
            _hook = _ntff_profile_via_ctypes("/opt/axon/libaxon_pjrt.so")
        except Exception:
            _hook = None
        mod.get_axon_ntff_profile_hook = lambda: _hook
        mod.set_axon_ntff_profile_hook = lambda h: None
        sys.modules["antenv.axon_hooks"] = mod

    import concourse.mybir as mybir
    import concourse.tile as tile
    from bass_rust import VectorClock, ScopedClock, N_PROCS

    if not getattr(tile.TileContext._add_instruction, "_split_waits_patch", False):
        # This walrus build accepts at most ONE sync wait per instruction
        # ("Too many sync wait commands"). Split extras onto single-wait
        # nops on the same engine, inserted immediately before. Stalling the
        # engine at the same program point is strictly stronger than the
        # per-instruction wait, and every waited-on producer is issued
        # earlier in program order, so this cannot deadlock.
        _orig_add = tile.TileContext._add_instruction

        def _add_instruction(self, inst):
            si = inst.sync_info
            if si is not None and si.on_wait and len(si.on_wait) > 1:
                waits = list(si.on_wait)
                for w in waits[:-1]:
                    nop = mybir.InstNoOp(
                        name=self.nc.get_next_instruction_name()
                    )
                    nop.engine = inst.engine
                    nop.sync_info = mybir.SyncInfo(on_wait=[w], on_update=[])
                    _orig_add(self, nop)
                inst.sync_info = mybir.SyncInfo(
                    on_wait=[waits[-1]], on_update=list(si.on_update or [])
                )
            _orig_add(self, inst)

        _add_instruction._split_waits_patch = True
        tile.TileContext._add_instruction = _add_instruction

    if not getattr(tile.TileContext._drain_and_barrier, "_split_waits_patch", False):
        # core_v3 walrus rejects a Drain carrying >1 sync wait ("Too many sync
        # wait commands"); put each wait on its own in-order SP nop instead.
        def _drain_and_barrier(self, tick_clock, wait_clock):
            nc = self.nc
            gc = tick_clock.global_clock
            for p in range(N_PROCS):
                t = gc[p]
                if t == 0:
                    continue
                vc = VectorClock([t if i == p else 0 for i in range(N_PROCS)])
                n = nc.sync.nop()
                wait_clock.add_sem_waits(n.ins, ScopedClock({None: vc}))
            nc.sync.drain()
            nc.all_engine_barrier()
            popped = nc._tile_sem_poison_stack.pop()
            assert popped is self._sem_poison
            nc.clear_and_free_semaphores(list(self.sems.allocated().values()))
            nc.all_engine_barrier()

        _drain_and_barrier._split_waits_patch = True
        tile.TileContext._drain_and_barrier = _drain_and_barrier

    from concourse import bass_utils
    bass_utils.upload_artifacts = lambda tmpdir: tmpdir
    _COMPAT_DONE = True


# ---------------------------------------------------------------------------
# host routing — bitwise replication of the reference (jax on CPU)
# ---------------------------------------------------------------------------
def _route(xt_f32, w_router):
    import jax
    import jax.numpy as jnp
    from jax import lax

    cpu = jax.devices("cpu")[0]

    def sinkhorn(cost, tol=1e-4):
        cost = jnp.exp(cost)
        T, E_ = cost.shape
        eps = 1e-8

        def cond(state):
            _, _, err = state
            return err > tol

        def body(state):
            d0, d1, _ = state
            d0n = (1.0 / T) / (cost @ d1 + eps)
            d1n = (1.0 / E_) / (d0n @ cost + eps)
            return (d0n, d1n, jnp.mean(jnp.abs(d1 - d1n)))

        init = (jnp.ones((T,), cost.dtype), jnp.ones((E_,), cost.dtype),
                jnp.asarray(1e9, cost.dtype))
        d0, d1, _ = lax.while_loop(cond, body, init)
        return d1 * cost * d0[:, None]

    with jax.default_device(cpu):
        xt_j = jnp.asarray(xt_f32)
        logits = xt_j @ jnp.asarray(w_router)
        norm = sinkhorn(logits.astype(jnp.float32))
        _, indices = lax.top_k(norm, 2)
        scores = jnp.take_along_axis(jax.nn.sigmoid(logits), indices, axis=1)
        return np.asarray(indices), np.asarray(scores)


# ---------------------------------------------------------------------------
# device kernel
# ---------------------------------------------------------------------------
_BUILD_CACHE = {}
LAST_EXEC_NS = None
LAST_RESULTS = None


def _build_nc(C):
    """Bass program for one core: dense FFN over C pre-gathered rows."""
    import concourse.bass as bass
    import concourse.mybir as mybir
    import concourse.tile as tile

    assert C % P == 0
    KC = H // P            # 4  k-chunks over hidden
    FC = F // P            # 16 f-chunks over ffn
    bf16 = mybir.dt.bfloat16
    f32 = mybir.dt.float32

    # token panels: full PANELs plus one remainder panel (multiple of 128)
    panels = []
    off = 0
    while off < C:
        w = min(PANEL, C - off)
        panels.append((off, w))
        off += w

    nc = bass.Bass()
    xt_d = nc.dram_tensor("xt", [P, KC, C], bf16, kind="ExternalInput")
    w1_d = nc.dram_tensor("w1", [P, KC, F], bf16, kind="ExternalInput")
    w2_d = nc.dram_tensor("w2", [P, FC, H], bf16, kind="ExternalInput")
    g_d = nc.dram_tensor("g", [P, C // P], f32, kind="ExternalInput")
    y_d = nc.dram_tensor("y", [C // P, P, H], f32, kind="ExternalOutput")

    with tile.TileContext(nc) as tc:
        with (
            tc.tile_pool(name="wpool", bufs=1) as wp,
            tc.tile_pool(name="xpool", bufs=3) as xp,
            tc.tile_pool(name="hpool", bufs=2) as hp,
            tc.tile_pool(name="opool", bufs=4) as op,
            tc.tile_pool(name="psum", bufs=4, space="PSUM") as pp,
        ):
            # first panel's tokens before the weights: PE needs x0 + w1 to
            # start; chunked DMAs land on parallel HW queues.
            x0_sb = xp.tile([P, KC, PANEL], bf16, tag="x")
            nc.sync.dma_start(out=x0_sb, in_=xt_d[:, :, 0:panels[0][1]])

            w1_sb = wp.tile([P, KC, F], bf16)
            for kc in range(KC):
                nc.sync.dma_start(out=w1_sb[:, kc, :], in_=w1_d[:, kc, :])
            w2_sb = wp.tile([P, FC, H], bf16)
            for q in range(4):
                nc.sync.dma_start(
                    out=w2_sb[:, q * 4:(q + 1) * 4, :],
                    in_=w2_d[:, q * 4:(q + 1) * 4, :],
                )
            g_sb = wp.tile([P, C // P], f32)
            nc.sync.dma_start(out=g_sb, in_=g_d[:, :])

            for ip, (poff, pw) in enumerate(panels):
                tch_n = pw // P
                if ip == 0:
                    x_sb = x0_sb
                else:
                    x_sb = xp.tile([P, KC, PANEL], bf16, tag="x")
                    nc.sync.dma_start(
                        out=x_sb[:, :, :pw], in_=xt_d[:, :, poff:poff + pw]
                    )
                h1_sb = hp.tile([P, FC, PANEL], bf16, tag="h1")
                for fc in range(FC):
                    ps = pp.tile([P, PANEL], f32, tag="ps1")
                    for kc in range(KC):
                        nc.tensor.matmul(
                            ps[:, :pw],
                            w1_sb[:, kc, fc * P:(fc + 1) * P],
                            x_sb[:, kc, :pw],
                            start=(kc == 0),
                            stop=(kc == KC - 1),
                        )
                    nc.scalar.activation(
                        out=h1_sb[:, fc, :pw], in_=ps[:, :pw],
                        func=mybir.ActivationFunctionType.Silu,
                    )
                for tch in range(tch_n):
                    ps2 = pp.tile([P, H], f32, tag="ps2")
                    for fc in range(FC):
                        nc.tensor.matmul(
                            ps2,
                            h1_sb[:, fc, tch * P:(tch + 1) * P],
                            w2_sb[:, fc, :],
                            start=(fc == 0),
                            stop=(fc == FC - 1),
                        )
                    o_sb = op.tile([P, H], f32, tag="o")
                    j = poff // P + tch
                    nc.vector.tensor_scalar_mul(o_sb, ps2, g_sb[:, j:j + 1])
                    nc.sync.dma_start(out=y_d[j], in_=o_sb)
    return nc


def _pack_core(xt_f32, toks, gates, w1_e_bf, w2_e_bf, C):
    n = len(toks)
    xr = np.zeros((C, H), _BF16)
    xr[:n] = xt_f32[toks].astype(_BF16)
    # [C,H] -> [H,C] -> [KC,P,C] -> [P,KC,C]
    xt_pack = np.ascontiguousarray(
        xr.T.reshape(H // P, P, C).transpose(1, 0, 2)
    )
    g = np.zeros((C,), np.float32)
    g[:n] = gates
    g_pack = np.ascontiguousarray(g.reshape(C // P, P).T)
    return {"xt": xt_pack, "w1": w1_e_bf, "w2": w2_e_bf, "g": g_pack}


def kernel(input, w_router, w1, w2):
    global LAST_EXEC_NS, LAST_RESULTS
    import os

    _install_compat()
    from concourse.bass_utils import run_bass_kernel_spmd

    x = np.asarray(input, dtype=np.float32)
    w_router = np.asarray(w_router, dtype=np.float32)
    w1 = np.asarray(w1, dtype=np.float32)
    w2 = np.asarray(w2, dtype=np.float32)
    s, b, h = x.shape
    T = s * b
    xt = np.ascontiguousarray(x.reshape(T, h))

    indices, scores = _route(xt, w_router)

    # per-expert (token, gate) lists
    tok_lists = []
    gate_lists = []
    for e in range(E):
        toks = []
        gs = []
        for k in range(2):
            sel = np.nonzero(indices[:, k] == e)[0]
            toks.append(sel)
            gs.append(scores[sel, k])
        tok_lists.append(np.concatenate(toks))
        gate_lists.append(np.concatenate(gs).astype(np.float32))

    max_half = max((len(t) + 1) // 2 for t in tok_lists)
    C = max(DEFAULT_C, ((max_half + P - 1) // P) * P)

    if C not in _BUILD_CACHE:
        _BUILD_CACHE[C] = _build_nc(C)
    nc = _BUILD_CACHE[C]

    # weights per expert, packed [P, KC, F] / [P, FC, H] bf16
    w1_packs = [
        np.ascontiguousarray(
            w1[e].astype(_BF16).reshape(H // P, P, F).transpose(1, 0, 2)
        )
        for e in range(E)
    ]
    w2_packs = [
        np.ascontiguousarray(
            w2[e].astype(_BF16).reshape(F // P, P, H).transpose(1, 0, 2)
        )
        for e in range(E)
    ]

    in_maps = []
    core_toks = []
    for c in range(NCORES):
        e = c // 2
        toks_e = tok_lists[e]
        gates_e = gate_lists[e]
        half = (len(toks_e) + 1) // 2
        if c % 2 == 0:
            toks, gs = toks_e[:half], gates_e[:half]
        else:
            toks, gs = toks_e[half:], gates_e[half:]
        assert len(toks) <= C
        core_toks.append(toks)
        in_maps.append(_pack_core(xt, toks, gs, w1_packs[e], w2_packs[e], C))

    trace = bool(int(os.environ.get("BASS_MOE_TRACE", "0")))
    res = run_bass_kernel_spmd(nc, in_maps, list(range(NCORES)), trace=trace)
    LAST_EXEC_NS = res.exec_time_ns
    LAST_RESULTS = res

    out = np.zeros((T, H), np.float32)
    for c in range(NCORES):
        y = res.results[c]["y"].reshape(-1, H)
        toks = core_toks[c]
        out[toks] += y[: len(toks)]
    return out.reshape(s, b, h)
